# revision 27
# baseline (speedup 1.0000x reference)
"""DifferentiableLengthRegulator Trainium2 kernel.

out[b,c,l] = y_mask * (sum_t x[b,c,t]*W[b,t,l]) / (sum_t W[b,t,l] + eps)
W = exp(-0.5*(l - center[b,t])^2 / (w[b,t]^2*sigma_scale^2 + eps))

Sharding: data-parallel over batch B=16 -> 8 cores x 2 batches.
Per core, per batch (banded over the frame axis, since the Gaussian
weights vanish outside ~13 sigma of each token chunk's centers):
  DVE : mu = pos - c                        (tensor_scalar, 2x fp32)
  ACT : W  = DerivErf(s*mu) -> bf16         (= 2/sqrt(pi) * exp(-(s*mu)^2);
        the 2/sqrt(pi) factor cancels in the normalization)
  PE  : psum[l,0:257] = sum_tc W_tc[:,l-slice]^T @ [xT | ones]  (bf16)
  DVE/ACT/POOL: rd = y_mask/(psum[:,256]+eps);
        out_sb[l,c] = psum[l,0:256]*rd (PSUM->SBUF move, engine-balanced)
Output written (B, L, C)-contiguous; host returns the transpose view.
"""

import numpy as np
import ml_dtypes

B, C, T, L = 16, 256, 512, 4096
N_CORES = 8
BPC = B // N_CORES  # batches per core
CH = 128            # partition chunk
TCN = T // CH       # 4 token chunks
LCN = L // CH       # 32 frame chunks
GRP = 4             # frame chunks per psum group
NGRP = LCN // GRP   # 8 groups
EPS = 1e-8
MARGIN_SIGMA = 13.19
BAND_ALIGN = 128

_bf16 = ml_dtypes.bfloat16
_cache = {}


def _center_scale(w, sigma_scale):
    """Mirror the reference's cumsum/center math (same jax backend bits)."""
    try:
        import jax.numpy as jnp

        wj = jnp.asarray(w)
        center = np.asarray(jnp.cumsum(wj, axis=1) - 0.5 * wj, dtype=np.float32)
    except Exception:
        center = (np.cumsum(w, axis=1, dtype=np.float32) - 0.5 * w).astype(np.float32)
    sigma = (w * np.float32(sigma_scale)).astype(np.float32)
    # W = DerivErf(s*mu)*sqrt(pi)/2 = exp(-(s*mu)^2), s = sqrt(0.5/(sig^2+eps))
    s = np.sqrt(np.float32(0.5) / (np.square(sigma) + np.float32(EPS))).astype(np.float32)
    return center, s


def _bands(center, w_all):
    """Per (slot, tc) aligned frame band, unioned across cores (SPMD)."""
    margin = float(MARGIN_SIGMA * w_all.max() + 1.0)
    bands = []
    for slot in range(BPC):
        rows = center[slot::BPC]  # the 8 batches that land in this slot
        sb = []
        for tc in range(TCN):
            seg = rows[:, tc * CH:(tc + 1) * CH]
            bs = max(0, int(np.floor((seg.min() - margin) / BAND_ALIGN)) * BAND_ALIGN)
            be = min(L, int(np.ceil((seg.max() + margin) / BAND_ALIGN)) * BAND_ALIGN)
            if tc == 0:
                bs = 0
            if tc == TCN - 1:
                be = L
            bs = min(bs, be - CH)
            sb.append((bs, be))
        bands.append(sb)
    return bands


def _split_excess_waits(nc, max_waits=1):
    """walrus here caps sync-waits at 1 per compute instruction; move the
    excess onto injected same-engine NoOps just before the instruction
    (waiting earlier on the same engine is always safe)."""
    from concourse import mybir

    for f in nc.m.functions:
        for blk in f.blocks:
            new = []
            for inst in blk.instructions:
                si = inst.sync_info
                if si is not None and len(si.on_wait) > max_waits:
                    waits = list(si.on_wait)
                    keep, extra = waits[-max_waits:], waits[:-max_waits]
                    for i in range(0, len(extra), max_waits):
                        nop = mybir.InstNoOp(name=f"{inst.name}-xw{i}", ins=[], outs=[])
                        nop.engine = inst.engine
                        nop.sync_info = mybir.SyncInfo(
                            on_wait=extra[i:i + max_waits], on_update=[])
                        new.append(nop)
                    inst.sync_info = mybir.SyncInfo(
                        on_wait=keep, on_update=list(si.on_update))
                new.append(inst)
            blk.instructions = new


def _build(band_key):
    import concourse.bass as bass
    import concourse.tile as tile
    from concourse import mybir

    bands = [[(band_key[s][t][0], band_key[s][t][1]) for t in range(TCN)]
             for s in range(BPC)]
    wmax = max(be - bs for sb in bands for (bs, be) in sb)

    nc = bass.Bass("TRN2", target_bir_lowering=False, debug=False)
    xta_d = nc.declare_dram_parameter("xta", [BPC, T, C + 1], mybir.dt.bfloat16, isOutput=False)
    pos_d = nc.declare_dram_parameter("pos", [CH, L], mybir.dt.float32, isOutput=False)
    coefs_d = nc.declare_dram_parameter("coefs", [3 * BPC * TCN, CH], mybir.dt.float32, isOutput=False)
    ym_d = nc.declare_dram_parameter("ym", [BPC * LCN, CH], mybir.dt.float32, isOutput=False)
    out_d = nc.declare_dram_parameter("out", [BPC, L, C], mybir.dt.float32, isOutput=True)

    f32 = mybir.dt.float32
    bf16 = mybir.dt.bfloat16
    FT = mybir.ActivationFunctionType
    OP = mybir.AluOpType

    def bcast(ap_col, n):
        return bass.AP(tensor=ap_col.tensor, offset=ap_col.offset,
                       ap=list(ap_col.ap) + [[0, n]])

    with tile.TileContext(nc) as tc_:
        import contextlib

        with contextlib.ExitStack() as ctx:
            consts = ctx.enter_context(tc_.tile_pool(name="consts", bufs=1))
            xta_p = ctx.enter_context(tc_.tile_pool(name="xta", bufs=2))
            mu_p = ctx.enter_context(tc_.tile_pool(name="mu", bufs=3))
            w_pools = [ctx.enter_context(tc_.tile_pool(name=f"w{t}", bufs=2)) for t in range(TCN)]
            psum_p = ctx.enter_context(tc_.tile_pool(name="ps", bufs=2, space="PSUM"))
            small_p = ctx.enter_context(tc_.tile_pool(name="small", bufs=6))
            out_p = ctx.enter_context(tc_.tile_pool(name="osb", bufs=4))

            # --- constants (coefs first: mu needs them; pos split across the
            # two HWDGE issue engines so W-gen starts ~4us earlier) ---
            coefs_sb = consts.tile([CH, 3 * BPC * TCN], f32)
            nc.sync.dma_start(out=coefs_sb, in_=coefs_d[:, :].rearrange("n p -> p n"))
            pos_f = consts.tile([CH, L], f32)
            q = L // 4
            for i in range(4):
                eng = nc.sync if i % 2 == 0 else nc.scalar
                eng.dma_start(out=pos_f[:, i * q:(i + 1) * q],
                              in_=pos_d[:, i * q:(i + 1) * q])
            ym_sb = consts.tile([CH, BPC * LCN], f32)
            nc.scalar.dma_start(out=ym_sb, in_=ym_d[:, :].rearrange("n p -> p n"))
            # W carries DerivErf's 2/sqrt(pi) factor; scaling eps by the same
            # factor makes rd = ym/(k*sumW + k*eps) = ym/k/(sumW + eps) exact.
            eps_sb = consts.tile([CH, 1], f32)
            nc.vector.memset(eps_sb, float(EPS) * 2.0 / np.pi ** 0.5)

            def col(tile_, idx):
                return tile_[:, idx:idx + 1]

            def cidx(q, b, t):
                return (q * BPC + b) * TCN + t

            xta_tiles = {}
            w_tiles = {}

            def load_xta(b):
                xta_sb = xta_p.tile([CH, TCN, C + 1], bf16)
                nc.sync.dma_start(
                    out=xta_sb,
                    in_=xta_d[b].rearrange("(t p) c -> p t c", p=CH),
                )
                for t in range(TCN):
                    # x_mask fold on GpSimd (broadcast multiply, x cols only)
                    nc.gpsimd.tensor_tensor(
                        out=xta_sb[:, t, :C], in0=xta_sb[:, t, :C],
                        in1=bcast(col(coefs_sb, cidx(2, b, t)), C),
                        op=OP.mult,
                    )
                xta_tiles[b] = xta_sb

            def wgen(b, t):
                bs, be = bands[b][t]
                bw = be - bs
                mu = mu_p.tile([CH, wmax], f32, tag="mu")
                nc.vector.tensor_scalar(
                    out=mu[:, :bw], in0=pos_f[:, bs:be],
                    scalar1=col(coefs_sb, cidx(0, b, t)), scalar2=None,
                    op0=OP.subtract,
                )
                wt = w_pools[t].tile([CH, wmax], bf16)
                # W = 2/sqrt(pi) * exp(-(s*mu)^2); constant cancels via rd
                nc.scalar.activation(
                    out=wt[:, :bw], in_=mu[:, :bw], func=FT.Derivative_Erf,
                    scale=col(coefs_sb, cidx(1, b, t)),
                )
                w_tiles[(b, t)] = wt

            def group(b, g):
                sb = bands[b]
                pgrp = psum_p.tile([CH, GRP, 512], f32, tag="pgrp")
                for k in range(GRP):
                    j = g * GRP + k
                    lo = j * CH
                    ctc = [t for t in range(TCN) if sb[t][0] <= lo and lo + CH <= sb[t][1]]
                    if not ctc:
                        nc.vector.memset(pgrp[:, k, :C + 1], 0.0)
                        continue
                    for i, t in enumerate(ctc):
                        off = lo - sb[t][0]
                        nc.tensor.matmul(
                            out=pgrp[:, k, :C + 1],
                            lhsT=w_tiles[(b, t)][:, off:off + CH],
                            rhs=xta_tiles[b][:, t, :],
                            start=(i == 0), stop=(i == len(ctc) - 1),
                        )
                dtmp = small_p.tile([CH, GRP], f32, tag="dtmp")
                # d + eps on ScalarE
                nc.scalar.activation(out=dtmp, in_=pgrp[:, :, C],
                                     func=FT.Identity, bias=eps_sb[:, 0:1])
                rd = small_p.tile([CH, GRP], f32, tag="rd")
                nc.vector.reciprocal(out=rd, in_=dtmp)
                # fold y_mask into rd on GpSimd
                nc.gpsimd.tensor_tensor(
                    out=rd, in0=rd,
                    in1=ym_sb[:, b * LCN + g * GRP: b * LCN + g * GRP + GRP],
                    op=OP.mult,
                )
                ogrp = out_p.tile([CH, GRP, C], f32)
                if (b * NGRP + g) % 4 < 3:
                    # normalize all 4 chunks in one DVE op (rd broadcast on a
                    # stride-0 free dim)
                    rdb = bass.AP(tensor=rd.tensor, offset=rd.offset,
                                  ap=[rd.ap[0], rd.ap[1], [0, C]])
                    nc.vector.tensor_tensor(
                        out=ogrp, in0=pgrp[:, :, :C], in1=rdb, op=OP.mult,
                    )
                else:
                    for k in range(GRP):
                        nc.scalar.activation(
                            out=ogrp[:, k, :], in_=pgrp[:, k, :C],
                            func=FT.Copy, scale=col(rd, k),
                        )
                nc.sync.dma_start(
                    out=out_d[b, g * GRP * CH:(g + 1) * GRP * CH, :].rearrange(
                        "(k p) c -> p k c", p=CH),
                    in_=ogrp,
                )

            # batch 0 weight phase, then interleave batch 1's weight phase
            # into batch 0's matmul/normalize groups to keep all engines fed.
            load_xta(0)
            wgen(0, 0)
            # PE warm-up on real data: ~20 back-to-back matmuls (~5us) bridge
            # the gap until the group stream starts, so HAM un-throttles and
            # the real matmuls run at 2.4GHz.
            wps = psum_p.tile([CH, GRP, 512], f32, tag="pgrp")
            for i in range(20):
                nc.tensor.matmul(
                    out=wps[:, 0, :C + 1], lhsT=w_tiles[(0, 0)][:, :CH],
                    rhs=xta_tiles[0][:, 0, :], start=True, stop=True,
                )
            for t in range(1, TCN):
                wgen(0, t)
            load_xta(1)
            for g in range(NGRP):
                group(0, g)
                if g in (0, 2, 4, 6):
                    wgen(1, g // 2)
            for g in range(NGRP):
                group(1, g)
    return nc


def _prepare_inputs(x, w, x_mask, y_mask, sigma_scale):
    center, s = _center_scale(w, sigma_scale[0])
    bands = _bands(center, w)

    xt = np.ascontiguousarray(x.transpose(0, 2, 1))          # (B, T, C)
    xta = np.concatenate([xt, np.ones((B, T, 1), np.float32)], axis=2)
    xta = xta.astype(_bf16)                                   # (B, T, C+1)

    xm = np.broadcast_to(x_mask.reshape(B, T), (B, T)).astype(np.float32)
    ymf = np.broadcast_to(y_mask.reshape(B, L), (B, L)).astype(np.float32)
    pos = np.broadcast_to(np.arange(L, dtype=np.float32), (CH, L)).copy()

    in_maps = []
    for core in range(N_CORES):
        bsel = [core * BPC + s_ for s_ in range(BPC)]
        coefs = np.empty((3, BPC, TCN, CH), np.float32)
        for s_, bb in enumerate(bsel):
            coefs[0, s_] = center[bb].reshape(TCN, CH)
            coefs[1, s_] = s[bb].reshape(TCN, CH)
            coefs[2, s_] = xm[bb].reshape(TCN, CH)
        ym_c = np.stack([ymf[bb].reshape(LCN, CH) for bb in bsel])  # (BPC,LCN,CH)
        in_maps.append({
            "xta": xta[bsel],
            "pos": pos,
            "coefs": coefs.reshape(3 * BPC * TCN, CH),
            "ym": ym_c.reshape(BPC * LCN, CH),
        })
    band_key = tuple(tuple(tuple(p) for p in sb) for sb in bands)
    return in_maps, band_key


def kernel(x, w, x_mask, y_mask, sigma_scale):
    x = np.asarray(x, dtype=np.float32)
    w = np.asarray(w, dtype=np.float32)
    x_mask = np.asarray(x_mask, dtype=np.float32)
    y_mask = np.asarray(y_mask, dtype=np.float32)
    sigma_scale = np.asarray(sigma_scale, dtype=np.float32)
    assert x.shape == (B, C, T) and w.shape == (B, T)

    in_maps, band_key = _prepare_inputs(x, w, x_mask, y_mask, sigma_scale)

    if band_key not in _cache:
        nc = _build(band_key)
        _split_excess_waits(nc)
        _cache[band_key] = nc
    nc = _cache[band_key]

    from concourse.bass_utils import run_bass_kernel_spmd

    res = run_bass_kernel_spmd(nc, in_maps, list(range(N_CORES)), trace=False)
    outs = [res.results[i]["out"] for i in range(N_CORES)]      # (BPC, L, C) each
    full = np.concatenate(outs, axis=0)                          # (B, L, C)
    return full.transpose(0, 2, 1)                               # (B, C, L)
